# revision 10
# baseline (speedup 1.0000x reference)
"""Trainium2 Bass kernel for nn_AblationGCN (2-layer OGB-style GCN).

Strategy: destination-node sharding. Nodes are bin-packed into 8 cores x B
blocks (<=128 nodes, <=K*128 incoming edges per block). Each core aggregates
messages for its own blocks with one-hot matmuls accumulating in PSUM:
    agg[slot, f] = sum_e S[e, slot] * hsrc[e, f],  S[e,slot]=(iota==slot)*norm
Edge-source rows are fetched with the GPSIMD dma_gather (Ant) instruction.
Because its indices are int16, sources are split into 4 windows of 25344
permuted slots; each block issues one gather call per window (trailing -1
indices are skipped by HW at no bandwidth cost). Between layers the per-core
h1 shards are AllGathered so every core can gather any source row. Degrees,
edge norms and the node permutation are computed host-side (index-space
preprocessing); all O(E*D)/O(N*D) feature math runs on device.
"""
import math
import numpy as np

import concourse.bass as bass
import concourse.bacc as bacc
import concourse.mybir as mybir
import concourse.tile as tile
from concourse.bass_utils import run_bass_kernel_spmd

P = 128
D = 128
NCORES = 8
LN_EPS = 1e-5
EDGE_CAP = 1024          # incoming-edge capacity per block (K=8 chunks)
NPASS = 4                # int16 source windows
GRP = 2                  # blocks per gather call (per pass); GRP*NIp must be <=1024
dt = mybir.dt

_CACHE = {}


# --------------------------------------------------------------------------
# Host-side planning: bin packing, permutation, per-core edge metadata
# --------------------------------------------------------------------------

def _pack_nodes(deg_in, nbins):
    """Pack nodes into nbins bins with <=128 nodes and <=EDGE_CAP in-edges.
    Returns bin id per node, or None if infeasible."""
    n = deg_in.shape[0]
    order = np.argsort(-deg_in, kind="stable")
    bin_load = np.zeros(nbins, np.int64)
    bin_cnt = np.zeros(nbins, np.int32)
    assign = np.full(n, -1, np.int32)
    import heapq
    heap = [(0, 0, b) for b in range(nbins)]
    heapq.heapify(heap)
    for v in order:
        d = int(deg_in[v])
        stash = []
        placed = False
        while heap:
            load, cnt, b = heapq.heappop(heap)
            if load != bin_load[b] or cnt != bin_cnt[b]:
                continue  # stale entry
            if cnt < P and load + d <= EDGE_CAP:
                assign[v] = b
                bin_load[b] += d
                bin_cnt[b] += 1
                if bin_cnt[b] < P:
                    heapq.heappush(heap, (int(bin_load[b]), int(bin_cnt[b]), b))
                placed = True
                break
            else:
                stash.append((load, cnt, b))
                # bins are popped cheapest-first; if the cheapest can't take
                # this node due to node-count, try the next ones
                if len(stash) > 64:
                    break
        for s in stash:
            heapq.heappush(heap, s)
        if not placed:
            return None
    return assign


def _plan(in_feat, edge_index, n, e, ncores):
    row = np.asarray(edge_index[0], dtype=np.int64)
    col = np.asarray(edge_index[1], dtype=np.int64)

    deg_math = np.bincount(row, minlength=n).astype(np.float64) + 1.0
    dis = deg_math ** -0.5
    norm_e = (dis[row] * dis[col]).astype(np.float32)
    deg_inv = (1.0 / deg_math).astype(np.float32)

    deg_in = np.bincount(col, minlength=n)

    nbins_min = max(
        math.ceil(n / (ncores * P)),
        math.ceil(e / (ncores * EDGE_CAP)),
    ) * ncores
    assign = None
    for nbins in range(nbins_min, nbins_min + 4 * ncores, ncores):
        assign = _pack_nodes(deg_in, nbins)
        if assign is not None:
            break
    assert assign is not None, "bin packing failed"
    B = nbins // ncores

    # order bins by load, snake-deal to cores for edge balance
    bin_load = np.bincount(assign, weights=deg_in.astype(np.float64),
                           minlength=nbins)
    order = np.argsort(-bin_load, kind="stable")
    bin_core = np.empty(nbins, np.int32)
    bin_local = np.empty(nbins, np.int32)
    cload = np.zeros(ncores, np.float64)
    ccnt = np.zeros(ncores, np.int32)
    for bid in order:
        c = int(np.argmin(cload))
        bin_core[bid] = c
        bin_local[bid] = ccnt[c]
        ccnt[c] += 1
        cload[c] += bin_load[bid]
    assert (ccnt == B).all()

    # slots within bins
    SLOTS = ncores * B * P
    perm_slot = np.full(n, -1, np.int64)
    nodes_sorted = np.lexsort((np.arange(n), assign))  # group nodes by bin
    # assign slot index within each bin in order
    slot_in_bin = np.zeros(n, np.int64)
    cnts = np.zeros(nbins, np.int64)
    for v in nodes_sorted:
        b = assign[v]
        slot_in_bin[v] = cnts[b]
        cnts[b] += 1
    perm_slot = (bin_core[assign].astype(np.int64) * (B * P)
                 + bin_local[assign].astype(np.int64) * P
                 + slot_in_bin)

    assert SLOTS % NPASS == 0
    wrows = SLOTS // NPASS  # rows per int16 source window
    assert wrows <= 32768

    src_slot = perm_slot[row]
    dst_core = bin_core[assign[col]]
    dst_block = bin_local[assign[col]]
    dst_slot = slot_in_bin[col]
    epass = src_slot // wrows

    # per (core, block, pass) run lengths -> uniform padded chunk counts C_p
    runs = np.zeros((ncores, B, NPASS), np.int64)
    np.add.at(runs, (dst_core, dst_block, epass), 1)
    C = [int(np.ceil(runs[:, :, p].max() / P)) for p in range(NPASS)]
    C = [max(c, 1) for c in C]
    NIp = [c * P for c in C]
    TOT = B * sum(NIp)          # padded edge slots per core per layer
    NCH = TOT // P              # matmul chunks per core per layer

    # order edges by (core, block, pass)
    ekey = np.lexsort((epass, dst_block, dst_core))
    r_s, nrm_s = src_slot[ekey], norm_e[ekey]
    dslot_s = dst_slot[ekey]
    dc_s, db_s, ep_s = dst_core[ekey], dst_block[ekey], epass[ekey]

    # Group-major padded layout: blocks are processed in groups of GRP; one
    # gather call covers (group, pass) = GRP consecutive blocks' runs, each
    # block's run padded to NIp[p] with valid idx-0 entries (norm 0) so that
    # -1 pads appear only at the very end of a call (max 127 of them).
    SNIv = int(sum(NIp))
    ngrp = (B + GRP - 1) // GRP
    gsz = [min(GRP, B - q * GRP) for q in range(ngrp)]
    goff = np.zeros(ngrp, np.int64)       # slot offset of group q
    for q in range(1, ngrp):
        goff[q] = goff[q - 1] + gsz[q - 1] * SNIv
    off_pass_g = []                        # per group: pass seg offsets
    for q in range(ngrp):
        ops = np.zeros(NPASS + 1, np.int64)
        for p in range(NPASS):
            ops[p + 1] = ops[p] + gsz[q] * NIp[p]
        off_pass_g.append(ops)

    qid = db_s // GRP
    gid = db_s % GRP
    grp_key = ((dc_s * B + db_s) * NPASS + ep_s)
    first = np.zeros(ncores * B * NPASS + 1, np.int64)
    np.add.at(first, grp_key + 1, 1)
    first = np.cumsum(first)
    rank = np.arange(e) - first[grp_key]
    NIp_a = np.asarray(NIp, np.int64)
    opg = np.stack([off_pass_g[int(q)] for q in range(ngrp)])  # [ngrp,NPASS+1]
    pos = (goff[qid] + opg[qid, ep_s] + gid * NIp_a[ep_s] + rank)

    idx16 = np.full((ncores, TOT), -1, np.int16)
    slotT = np.zeros((ncores, TOT), np.float32)
    normT = np.zeros((ncores, TOT), np.float32)
    idx16[dc_s, pos] = (r_s - ep_s * wrows).astype(np.int16)
    slotT[dc_s, pos] = dslot_s.astype(np.float32)
    normT[dc_s, pos] = nrm_s

    # fill pads: non-final blocks of each call to exactly NIp[p]; the final
    # block to at least NIp[p]-127 (valid idx 0, norm stays 0)
    for c in range(ncores):
        for q in range(ngrp):
            for p in range(NPASS):
                for g in range(gsz[q]):
                    b = q * GRP + g
                    v = int(runs[c, b, p])
                    tgt = NIp[p] if g < gsz[q] - 1 else max(NIp[p] - 127, 1)
                    if v < tgt:
                        base = int(goff[q] + opg[q, p] + g * NIp_a[p])
                        idx16[c, base + v:base + tgt] = 0

    # wrapped idx layout per call (call = group x pass span)
    idxw = np.zeros((ncores, P, TOT // 16), np.int16)
    for c in range(ncores):
        flat = idx16[c]
        for q in range(ngrp):
            for p in range(NPASS):
                ni = int(gsz[q] * NIp[p])
                base = int(goff[q] + opg[q, p])
                seg = flat[base:base + ni]
                w = seg.reshape(ni // 16, 16).T  # [16, ni/16]
                cb0 = base // 16
                for g8 in range(8):
                    idxw[c, g8 * 16:(g8 + 1) * 16, cb0:cb0 + ni // 16] = w

    # per-chunk transposed metadata [128, NCH]
    slotTw = slotT.reshape(ncores, NCH, P).transpose(0, 2, 1).copy()
    normTw = normT.reshape(ncores, NCH, P).transpose(0, 2, 1).copy()

    # deg_inv per (core, block, slot) + node ids for unpermute
    deginvT = np.zeros((ncores, P, B), np.float32)
    node_of = np.full((ncores, B * P), -1, np.int64)
    allv = np.arange(n)
    cc = bin_core[assign[allv]]
    bb = bin_local[assign[allv]]
    ss = slot_in_bin[allv]
    deginvT[cc, ss, bb] = deg_inv[allv]
    node_of[cc, bb * P + ss] = allv

    return dict(
        B=B, C=C, NIp=NIp, TOT=TOT, NCH=NCH, SLOTS=SLOTS, wrows=wrows,
        idxw=idxw, slotTw=slotTw, normTw=normTw, deginvT=deginvT,
        node_of=node_of, perm_slot=perm_slot, ngrp=ngrp, gsz=gsz,
        goff=[int(x) for x in goff], opg=[[int(x) for x in r] for r in opg],
    )


# --------------------------------------------------------------------------
# Device program
# --------------------------------------------------------------------------

def _build(B, C, NIp, TOT, NCH, SLOTS, ncores, ngrp=None, gsz=None,
           goff=None, opg=None, gbufs=2, rep=1):
    nc = bacc.Bacc("TRN2", target_bir_lowering=False, debug=False,
                   num_devices=ncores, num_swdge_queues=4)
    SH = B * P  # shard rows
    CW = P + 2 * NCH + B + 6 * P + 1  # iota | slotT | normT | deginv | 6 bcast | eps

    ifr = nc.dram_tensor("ifr", [SLOTS, D], dt.float32, kind="ExternalInput")
    rootf = nc.dram_tensor("rootf", [SH, D], dt.float32, kind="ExternalInput")
    idx16 = nc.dram_tensor("idx16", [P, TOT // 16], dt.int16,
                           kind="ExternalInput")
    cstF = nc.dram_tensor("cstF", [P, CW], dt.float32, kind="ExternalInput")
    out_sh = nc.dram_tensor("out_sh", [SH, D], dt.float32,
                            kind="ExternalOutput")

    SNI = int(sum(NIp))
    CSUM = [int(x) for x in np.concatenate([[0], np.cumsum(NIp)])]

    with tile.TileContext(nc) as tc:
        with (
            tc.tile_pool(name="const", bufs=1) as cpool,
            tc.tile_pool(name="gbuf", bufs=gbufs) as gpool,
            tc.tile_pool(name="spool", bufs=4) as spool,
            tc.tile_pool(name="fpool", bufs=3) as fpool,
            tc.tile_pool(name="small", bufs=4) as mpool,
            tc.tile_pool(name="psum", bufs=2, space="PSUM") as psum,
            tc.tile_pool(name="dram", bufs=1, space="DRAM") as dram,
        ):
            h1_sh = dram.tile([SH, D], dt.float32)
            h1_full = dram.tile([SLOTS, D], dt.float32)

            cb = cpool.tile([P, CW], dt.float32)
            ix = cpool.tile([P, TOT // 16], dt.int16)
            nc.sync.dma_start(out=cb[:], in_=cstF[:])
            nc.sync.dma_start(out=ix[:], in_=idx16[:])
            iota = cb[:, 0:P]
            o_slot = P
            o_norm = P + NCH
            o_dinv = P + 2 * NCH
            o_bc = P + 2 * NCH + B  # emb0|emb1|g0|b0|g1|b1
            o_eps = o_bc + 6 * P

            call_no = 0

            def layer(src_dram, root_dram, li, out_dram):
                nonlocal call_no
                emb = cb[:, o_bc + (0 if li == 0 else P):
                         o_bc + (0 if li == 0 else P) + P]
                g_ = cb[:, o_bc + 2 * P + (0 if li == 0 else 2 * P):
                        o_bc + 3 * P + (0 if li == 0 else 2 * P)]
                b_ = cb[:, o_bc + 3 * P + (0 if li == 0 else 2 * P):
                        o_bc + 4 * P + (0 if li == 0 else 2 * P)]
                wrows = SLOTS // NPASS
                for q in range(ngrp):
                    G = gsz[q]
                    gts = []
                    for p in range(NPASS):
                        ni = G * NIp[p]
                        gt = gpool.tile([P, GRP * NIp[p]], dt.float32,
                                        tag=f"gt{p}")
                        if li == 0 and q < gbufs:
                            nc.vector.memset(gt[:], 0.0)
                        base = goff[q] + opg[q][p]
                        nc.gpsimd.dma_gather(
                            out_ap=gt[:, 0:ni].rearrange(
                                "p (n d) -> p n d", d=D),
                            in_ap=src_dram[p * wrows:(p + 1) * wrows, :],
                            idxs_ap=ix[:, base // 16:base // 16 + ni // 16],
                            num_idxs=ni,
                            num_idxs_reg=ni,
                            elem_size=D,
                            queue_num=call_no % 4,
                        )
                        call_no += 1
                        gts.append(gt)
                    pss = [psum.tile([P, D], dt.float32, space="PSUM",
                                     tag=f"ps{g}", name=f"ps{g}")
                           for g in range(G)]
                    for p in range(NPASS):
                        for g in range(G):
                            for c in range(C[p]):
                                ci = ((goff[q] + opg[q][p]) // P
                                      + g * C[p] + c)
                                st = spool.tile([P, P], dt.float32, tag="st")
                                nc.vector.tensor_scalar(
                                    out=st[:], in0=iota,
                                    scalar1=cb[:, o_slot + ci:o_slot + ci + 1],
                                    scalar2=cb[:, o_norm + ci:o_norm + ci + 1],
                                    op0=mybir.AluOpType.is_equal,
                                    op1=mybir.AluOpType.mult,
                                )
                                nc.tensor.matmul(
                                    out=pss[g][:], lhsT=st[:],
                                    rhs=gts[p][:, (g * C[p] + c) * P:
                                               (g * C[p] + c + 1) * P],
                                    start=(p == 0 and c == 0),
                                    stop=(p == NPASS - 1 and c == C[p] - 1),
                                )
                    for g in range(G):
                        b = q * GRP + g
                        ps = pss[g]
                        _finalize(li, b, ps)

            def _finalize(li, b, ps):
                    emb = cb[:, o_bc + (0 if li == 0 else P):
                             o_bc + (0 if li == 0 else P) + P]
                    g_ = cb[:, o_bc + 2 * P + (0 if li == 0 else 2 * P):
                            o_bc + 3 * P + (0 if li == 0 else 2 * P)]
                    b_ = cb[:, o_bc + 3 * P + (0 if li == 0 else 2 * P):
                            o_bc + 4 * P + (0 if li == 0 else 2 * P)]
                    root_dram = rootf if li == 0 else h1_sh
                    out_dram = h1_sh if li == 0 else out_sh
                    # root term + LN
                    rf = fpool.tile([P, D], dt.float32, tag="rf")
                    nc.sync.dma_start(out=rf[:],
                                      in_=root_dram[b * P:(b + 1) * P, :])
                    t1 = fpool.tile([P, D], dt.float32, tag="t1")
                    nc.vector.tensor_tensor(out=t1[:], in0=rf[:], in1=emb,
                                            op=mybir.AluOpType.add)
                    t2 = fpool.tile([P, D], dt.float32, tag="t2")
                    nc.scalar.activation(t2[:], t1[:],
                                         mybir.ActivationFunctionType.Relu)
                    x = fpool.tile([P, D], dt.float32, tag="x")
                    nc.vector.tensor_scalar(
                        out=x[:], in0=t2[:],
                        scalar1=cb[:, o_dinv + b:o_dinv + b + 1],
                        scalar2=None, op0=mybir.AluOpType.mult)
                    nc.vector.tensor_tensor(out=x[:], in0=x[:], in1=ps[:],
                                            op=mybir.AluOpType.add)
                    mu = mpool.tile([P, 1], dt.float32, tag="mu")
                    nc.vector.reduce_sum(mu[:], x[:], axis=mybir.AxisListType.X)
                    nmu = mpool.tile([P, 1], dt.float32, tag="nmu")
                    nc.vector.tensor_scalar(out=nmu[:], in0=mu[:],
                                            scalar1=-1.0 / D, scalar2=None,
                                            op0=mybir.AluOpType.mult)
                    xc = fpool.tile([P, D], dt.float32, tag="xc")
                    nc.vector.tensor_scalar(out=xc[:], in0=x[:],
                                            scalar1=nmu[:, 0:1], scalar2=None,
                                            op0=mybir.AluOpType.add)
                    sq = fpool.tile([P, D], dt.float32, tag="sq")
                    ssq = mpool.tile([P, 1], dt.float32, tag="ssq")
                    nc.scalar.activation(sq[:], xc[:],
                                         mybir.ActivationFunctionType.Square,
                                         accum_out=ssq[:])
                    std = mpool.tile([P, 1], dt.float32, tag="std")
                    nc.scalar.activation(std[:], ssq[:],
                                         mybir.ActivationFunctionType.Sqrt,
                                         bias=cb[:, o_eps:o_eps + 1],
                                         scale=1.0 / D)
                    rstd = mpool.tile([P, 1], dt.float32, tag="rstd")
                    nc.vector.reciprocal(rstd[:], std[:])
                    y = fpool.tile([P, D], dt.float32, tag="y")
                    nc.vector.tensor_scalar(out=y[:], in0=xc[:],
                                            scalar1=rstd[:, 0:1], scalar2=None,
                                            op0=mybir.AluOpType.mult)
                    nc.vector.tensor_tensor(out=y[:], in0=y[:], in1=g_,
                                            op=mybir.AluOpType.mult)
                    nc.vector.tensor_tensor(out=y[:], in0=y[:], in1=b_,
                                            op=mybir.AluOpType.add)
                    if li == 0:
                        yr = fpool.tile([P, D], dt.float32, tag="yr")
                        nc.scalar.activation(yr[:], y[:],
                                             mybir.ActivationFunctionType.Relu)
                        y = yr
                    nc.sync.dma_start(out=out_dram[b * P:(b + 1) * P, :],
                                      in_=y[:])

            for _ in range(rep):
                layer(ifr, rootf, 0, h1_sh)
                nc.gpsimd.collective_compute(
                    "AllGather", mybir.AluOpType.bypass,
                    replica_groups=[list(range(ncores))],
                    ins=[h1_sh.opt()], outs=[h1_full.opt()],
                )
                layer(h1_full, h1_sh, 1, out_sh)
    nc.finalize()
    return nc


# --------------------------------------------------------------------------
# Entry points
# --------------------------------------------------------------------------

def prepare(in_feat, edge_index, root_emb0, root_emb1,
            ln0_g, ln0_b, ln1_g, ln1_b, ncores=NCORES, rep=1):
    in_feat = np.asarray(in_feat, dtype=np.float32)
    edge_index = np.asarray(edge_index)
    n, d = in_feat.shape
    e = edge_index.shape[1]
    assert d == D

    pl = _plan(in_feat, edge_index, n, e, ncores)
    B, NIp, TOT, NCH, SLOTS = pl["B"], pl["NIp"], pl["TOT"], pl["NCH"], pl["SLOTS"]

    key = (B, tuple(pl["C"]), TOT, SLOTS, ncores, rep, pl["ngrp"])
    if key not in _CACHE:
        _CACHE[key] = _build(B, pl["C"], NIp, TOT, NCH, SLOTS, ncores,
                             ngrp=pl["ngrp"], gsz=pl["gsz"],
                             goff=pl["goff"], opg=pl["opg"], rep=rep)
    nc = _CACHE[key]

    # gather source for layer 0: relu(in_feat) in permuted slot order
    ifr = np.zeros((SLOTS, D), np.float32)
    ifr[pl["perm_slot"]] = np.maximum(in_feat, 0.0)

    emb0 = np.broadcast_to(np.asarray(root_emb0, np.float32).reshape(1, D),
                           (P, D))
    emb1 = np.broadcast_to(np.asarray(root_emb1, np.float32).reshape(1, D),
                           (P, D))
    g0 = np.broadcast_to(np.asarray(ln0_g, np.float32).reshape(1, D), (P, D))
    b0 = np.broadcast_to(np.asarray(ln0_b, np.float32).reshape(1, D), (P, D))
    g1 = np.broadcast_to(np.asarray(ln1_g, np.float32).reshape(1, D), (P, D))
    b1 = np.broadcast_to(np.asarray(ln1_b, np.float32).reshape(1, D), (P, D))

    in_maps = []
    for c in range(ncores):
        node_of = pl["node_of"][c]
        rootf = np.zeros((B * P, D), np.float32)
        valid = node_of >= 0
        rootf[valid] = in_feat[node_of[valid]]
        cst = np.concatenate([
            np.tile(np.arange(P, dtype=np.float32), (P, 1)),
            pl["slotTw"][c], pl["normTw"][c], pl["deginvT"][c],
            emb0, emb1, g0, b0, g1, b1,
            np.full((P, 1), LN_EPS, np.float32),
        ], axis=1).astype(np.float32)
        in_maps.append({
            "ifr": ifr, "rootf": rootf, "idx16": pl["idxw"][c], "cstF": cst,
        })

    def post(results):
        out = np.zeros((n, D), np.float32)
        for c in range(ncores):
            node_of = pl["node_of"][c]
            valid = node_of >= 0
            out[node_of[valid]] = results[c]["out_sh"][valid]
        return out

    return nc, in_maps, post


def kernel(in_feat, edge_index, root_emb0, root_emb1,
           ln0_g, ln0_b, ln1_g, ln1_b):
    nc, in_maps, post = prepare(in_feat, edge_index, root_emb0, root_emb1,
                                ln0_g, ln0_b, ln1_g, ln1_b)
    res = run_bass_kernel_spmd(nc, in_maps, core_ids=list(range(NCORES)))
    return post(res.results)
